# revision 12
# baseline (speedup 1.0000x reference)
"""AsyncIOPool distributed Bass kernel for 8 TRN2 NeuronCores.

Problem: src[N,D], push_src[B,D], dst[N,D], index[B], dst_index[B]
out = concat(src[index], dst.at[dst_index].set(push_src))  -> [B+N, D]
N=500000, B=131072, D=256.

Sharding (host side, inside kernel()):
 - src replicated to all cores; gather indices sliced per core (BS = B/8)
 - dst row-sharded (NS = N/8 rows per core); out row-sharded to match
 - push rows routed to their owner shard (per the sharding hint), grouped
   into per-copy-chunk buckets, padded to fixed capacity with OOB indices
   (skipped on device via bounds_check)

Device (per core), engines:
 - gpsimd (SWDGE): nothing but the indirect DMAs (the Q7 descriptor
   generator is the critical resource): 128-row indirect gathers from the
   replicated src, then 128-row indirect scatters of push rows into out.
   HW limit: one gathered/scattered row per SBUF partition per call.
   Index tiles are semaphore-gated (SWDGE reads index values at
   descriptor-generation time).
 - sync (HWDGE): index-tile loads first (ring FIFO -> they land in a few
   us), then the bulk DRAM->DRAM copy of the dst shard into out, paced to
   gather progress so copy descriptors don't flood the SDMA rings ahead of
   the latency-sensitive SWDGE completions; chunk j incs its own semaphore
   so scatter bucket j can start as soon as chunk j landed.
 - scalar (HWDGE): batched stores of gathered rows + push-row bucket
   loads, self-paced via semaphores (kept off the copy ring so they do not
   queue behind 64MB of copy descriptors).

Semaphore discipline: every wait_ge threshold equals the TOTAL possible
increments of that semaphore from all its uses up to the awaited producer,
and rotation spacing guarantees no two uses of one semaphore are in flight
concurrently. (A lower threshold is racy: increments from later DMAs can
substitute for missing increments of earlier ones.)
"""
import numpy as np
import concourse.bass as bass
from concourse import bacc, mybir
from concourse.bass_utils import run_bass_kernel_spmd

# Optional: register the NTFF profile hook if the boot couldn't (lets
# BASS_TRACE / trace=True produce exec_time_ns under axon).
def _ensure_profile_hook():
    import sys, types
    if 'antenv.axon_hooks' in sys.modules:
        return
    try:
        from trn_agent_boot.trn_boot import _ntff_profile_via_ctypes
        hook = _ntff_profile_via_ctypes('/opt/axon/libaxon_pjrt.so')
    except Exception:
        return
    mod = types.ModuleType('antenv.axon_hooks')
    mod.get_axon_ntff_profile_hook = lambda: hook
    mod.set_axon_ntff_profile_hook = lambda h: None
    sys.modules['antenv.axon_hooks'] = mod

_ensure_profile_hook()


class Cfg:
    def __init__(self, N=500_000, B=131_072, D=256, NCORES=8,
                 N_BUCKETS=10, CAP=1792, STORE_BATCH=8,
                 SLOT_BATCHES=10, N_PBUFS=5, interleave=True):
        self.N, self.B, self.D, self.NCORES = N, B, D, NCORES
        self.BS = B // NCORES                 # gather rows per core
        self.NS = N // NCORES                 # dst rows per core
        self.N_BUCKETS = N_BUCKETS            # copy chunks == scatter buckets
        self.CHUNK = self.NS // N_BUCKETS     # dst rows per copy chunk
        assert self.NS % N_BUCKETS == 0
        self.CAP = CAP                        # push-row capacity per bucket
        assert CAP % 128 == 0
        self.SC_CALLS = CAP // 128            # scatter calls per bucket
        self.G_CALLS = self.BS // 128         # gather calls per core
        self.STORE_BATCH = STORE_BATCH        # gather calls per store
        self.SLOT_BATCHES = SLOT_BATCHES      # gbuf slot window, in batches
        self.N_SLOTS = SLOT_BATCHES * STORE_BATCH
        assert self.G_CALLS % STORE_BATCH == 0
        self.N_STORES = self.G_CALLS // STORE_BATCH
        self.N_PBUFS = N_PBUFS                # push-row SBUF buffers
        self.OOB = 1 << 20
        self.interleave = interleave          # scatter bucket j after copy j


def build(cfg):
    c = cfg
    f32, i32 = mybir.dt.float32, mybir.dt.int32
    nc = bacc.Bacc("TRN2", target_bir_lowering=False, debug=False,
                   num_devices=c.NCORES)

    src = nc.dram_tensor("src", [c.N, c.D], f32, kind="ExternalInput")
    gidx = nc.dram_tensor("gidx", [128, c.G_CALLS], i32, kind="ExternalInput")
    dsts = nc.dram_tensor("dsts", [c.NS, c.D], f32, kind="ExternalInput")
    prow = nc.dram_tensor("prow", [c.N_BUCKETS * c.CAP, c.D], f32,
                          kind="ExternalInput")
    pidx = nc.dram_tensor("pidx", [128, c.N_BUCKETS * c.SC_CALLS], i32,
                          kind="ExternalInput")
    out = nc.dram_tensor("out", [c.BS + c.NS, c.D], f32, kind="ExternalOutput")

    gidx_t = nc.alloc_sbuf_tensor("gidx_t", [128, c.G_CALLS], i32)
    pidx_t = nc.alloc_sbuf_tensor("pidx_t", [128, c.N_BUCKETS * c.SC_CALLS], i32)
    gbuf = nc.alloc_sbuf_tensor("gbuf", [128, c.N_SLOTS, c.D], f32)
    pbufs = [nc.alloc_sbuf_tensor(f"pbuf{i}", [128, c.SC_CALLS, c.D], f32)
             for i in range(c.N_PBUFS)]

    idx_sem = nc.alloc_semaphore("idx_sem")
    gsems = [nc.alloc_semaphore(f"gsem{i}") for i in range(12)]
    stsems = [nc.alloc_semaphore(f"stsem{i}") for i in range(4)]
    ldsems = [nc.alloc_semaphore(f"ldsem{i}") for i in range(c.N_PBUFS)]
    scsems = [nc.alloc_semaphore(f"scsem{i}") for i in range(c.N_PBUFS)]
    cp_sems = [nc.alloc_semaphore(f"cp_sem{j}") for j in range(c.N_BUCKETS)]

    M, P = c.SLOT_BATCHES, c.N_PBUFS
    rows_per_store = 128 * c.STORE_BATCH

    with nc.Block() as block:

        # sync: index tiles wait, then ALL bulk copies (dedicated ring so
        # stores/loads on scalar don't queue behind 64MB of copy descriptors)
        @block.sync
        def _(sync):
            # index tiles first: the ring is FIFO, so these 144KB land in a
            # few us before any copy descriptor queues behind them
            sync.dma_start(out=gidx_t.ap()[:], in_=gidx.ap()[:]).then_inc(idx_sem, 16)
            sync.dma_start(out=pidx_t.ap()[:], in_=pidx.ap()[:]).then_inc(idx_sem, 16)
            # pace copy issue to gather progress so the 64MB of copy
            # descriptors don't flood the SDMA rings ahead of the latency-
            # sensitive SWDGE work (2-chunk head start, then 1 chunk per 2
            # completed gather batches)
            for j in range(c.N_BUCKETS):
                if j >= 2 and c.N_STORES > 4:
                    b = min((j - 2) * 2, c.N_STORES - 1)
                    sync.wait_ge(gsems[b % 12],
                                 16 * c.STORE_BATCH * (b // 12 + 1))
                sync.dma_start(
                    out=out.ap()[c.BS + j * c.CHUNK: c.BS + (j + 1) * c.CHUNK, :],
                    in_=dsts.ap()[j * c.CHUNK: (j + 1) * c.CHUNK, :],
                ).then_inc(cp_sems[j], 16)

        # scalar: index tiles first (gate the gpsimd), then gather-batch
        # stores and push-row bucket loads, self-paced via semaphores
        @block.scalar
        def _(scalar):
            def store_batch(b):
                # all gathers of batch b complete (max threshold of gsems[b%6])
                scalar.wait_ge(gsems[b % 12], 16 * c.STORE_BATCH * (b // 12 + 1))
                s0 = (b % M) * c.STORE_BATCH
                scalar.dma_start(
                    out=out.ap()[b * rows_per_store: (b + 1) * rows_per_store, :]
                        .rearrange("(kk p) d -> p kk d", p=128),
                    in_=gbuf.ap()[:, s0: s0 + c.STORE_BATCH, :],
                ).then_inc(stsems[b % 4], 16)

            def load_bucket(j):
                if j >= P:
                    # pbuf reuse: scatters of bucket j-P must have read pbuf
                    scalar.wait_ge(scsems[(j - P) % P],
                                   16 * c.SC_CALLS * ((j - P) // P + 1))
                scalar.dma_start(
                    out=pbufs[j % P].ap()[:],
                    in_=prow.ap()[j * c.CAP: (j + 1) * c.CAP, :],
                ).then_inc(ldsems[j % P], 16)

            # order matters: loads with scsems waits (j >= P) must come after
            # every store, else a blocked load would stall stores the gather
            # stream needs (deadlock). Wait-free loads (j < P) go early.
            nst_head = min(6, c.N_STORES)
            for b in range(nst_head):
                store_batch(b)
            for j in range(min(P, c.N_BUCKETS)):
                load_bucket(j)
            for b in range(nst_head, c.N_STORES):
                store_batch(b)
            for j in range(min(P, c.N_BUCKETS), c.N_BUCKETS):
                load_bucket(j)

        # gpsimd: nothing but indirect DMAs (the Q7 descriptor generator is
        # the critical path) -- 128-row gathers, then 128-row scatters
        @block.gpsimd
        def _(gpsimd):
            # both index tiles fully loaded (max threshold -> race-free)
            gpsimd.wait_ge(idx_sem, 32)

            for k in range(c.G_CALLS):
                b = k // c.STORE_BATCH
                if k % c.STORE_BATCH == 0 and b >= M:
                    # slot reuse: store of batch b-M must have drained gbuf
                    gpsimd.wait_ge(stsems[(b - M) % 4],
                                   16 * ((b - M) // 4 + 1))
                gpsimd.indirect_dma_start(
                    out=gbuf.ap()[:, k % c.N_SLOTS, :],
                    out_offset=None,
                    in_=src.ap()[:],
                    in_offset=bass.IndirectOffsetOnAxis(
                        ap=gidx_t.ap()[:, k: k + 1], axis=0),
                ).then_inc(gsems[b % 12], 16)

            for j in range(c.N_BUCKETS):
                if c.interleave:
                    gpsimd.wait_ge(cp_sems[j], 16)
                elif j == 0:
                    for jj in range(c.N_BUCKETS):
                        gpsimd.wait_ge(cp_sems[jj], 16)
                # bucket j's push rows landed in pbuf (max threshold)
                gpsimd.wait_ge(ldsems[j % P], 16 * (j // P + 1))
                for s in range(c.SC_CALLS):
                    jc = j * c.SC_CALLS + s
                    gpsimd.indirect_dma_start(
                        out=out.ap()[:],
                        out_offset=bass.IndirectOffsetOnAxis(
                            ap=pidx_t.ap()[:, jc: jc + 1], axis=0),
                        in_=pbufs[j % P].ap()[:, s, :],
                        in_offset=None,
                        element_offset=c.BS * c.D,
                        bounds_check=c.NS - 1,
                        oob_is_err=False,
                    ).then_inc(scsems[j % P], 16)

    nc.compile()
    return nc


def shard_inputs(cfg, src, push_src, dst, index, dst_index):
    """Host-side sharding/routing -> in_maps for run_bass_kernel_spmd."""
    c = cfg
    src = np.ascontiguousarray(np.asarray(src, dtype=np.float32))
    push_src = np.ascontiguousarray(np.asarray(push_src, dtype=np.float32))
    dst = np.asarray(dst, dtype=np.float32)
    index = np.asarray(index).astype(np.int64, copy=False)
    dst_index = np.asarray(dst_index).astype(np.int64, copy=False)

    owner = dst_index // c.NS
    local_all = (dst_index - owner * c.NS).astype(np.int32)

    in_maps = []
    for i in range(c.NCORES):
        gidx2d = np.ascontiguousarray(
            index[i * c.BS:(i + 1) * c.BS].astype(np.int32)
            .reshape(c.G_CALLS, 128).T)

        m = owner == i
        pos = np.nonzero(m)[0]
        loc = local_all[pos]
        bkt = loc // c.CHUNK
        order = np.argsort(bkt, kind="stable")
        pos, loc, bkt = pos[order], loc[order], bkt[order]
        counts = np.bincount(bkt, minlength=c.N_BUCKETS)

        prow = np.zeros((c.N_BUCKETS * c.CAP, c.D), np.float32)
        pidx = np.full((c.N_BUCKETS * c.CAP,), c.OOB, np.int32)
        dsts_i = dst[i * c.NS:(i + 1) * c.NS]
        dsts_copied = False
        start = 0
        for j in range(c.N_BUCKETS):
            cnt = int(counts[j])
            take = min(cnt, c.CAP)
            prow[j * c.CAP: j * c.CAP + take] = push_src[pos[start:start + take]]
            pidx[j * c.CAP: j * c.CAP + take] = loc[start:start + take]
            if cnt > take:  # capacity overflow: pre-merge the tail on host
                if not dsts_copied:
                    dsts_i = dsts_i.copy()
                    dsts_copied = True
                ov = slice(start + take, start + cnt)
                dsts_i[loc[ov]] = push_src[pos[ov]]
            start += cnt

        pidx2d = np.ascontiguousarray(
            pidx.reshape(c.N_BUCKETS, 128, c.SC_CALLS)
            .transpose(1, 0, 2).reshape(128, c.N_BUCKETS * c.SC_CALLS))

        in_maps.append({
            "src": src,
            "gidx": gidx2d,
            "dsts": np.ascontiguousarray(dsts_i),
            "prow": prow,
            "pidx": pidx2d,
        })
    return in_maps


def unshard(cfg, results):
    c = cfg
    full = np.empty((c.B + c.N, c.D), np.float32)
    for i in range(c.NCORES):
        o = results[i]["out"]
        full[i * c.BS:(i + 1) * c.BS] = o[:c.BS]
        full[c.B + i * c.NS: c.B + (i + 1) * c.NS] = o[c.BS:]
    return full


_CFG = Cfg()
_NC = None


def _get_nc():
    global _NC
    if _NC is None:
        _NC = build(_CFG)
    return _NC


def kernel(src, push_src, dst, index, dst_index):
    nc = _get_nc()
    in_maps = shard_inputs(_CFG, src, push_src, dst, index, dst_index)
    res = run_bass_kernel_spmd(nc, in_maps,
                               core_ids=list(range(_CFG.NCORES)))
    return unshard(_CFG, res.results)


def kernel_profiled(src, push_src, dst, index, dst_index):
    """Like kernel() but with NTFF tracing; returns (out, exec_time_ns)."""
    nc = _get_nc()
    in_maps = shard_inputs(_CFG, src, push_src, dst, index, dst_index)
    res = run_bass_kernel_spmd(nc, in_maps,
                               core_ids=list(range(_CFG.NCORES)), trace=True)
    return unshard(_CFG, res.results), res.exec_time_ns
